# revision 1
# baseline (speedup 1.0000x reference)
"""Two-layer SAGEConv(mean) + PReLU GNN encoder on 8 Trainium2 NeuronCores.

Strategy (matches the sharding hint): partition nodes (and their incoming
edges) across the 8 cores; build the full source-feature table per layer via
AllGather; gather source rows with dma_gather; segment-sum via one-hot matmul
into per-group PSUM; fused epilogue (mean scale, two 64x64 matmuls + bias,
PReLU) per 128-node group.

Host-side prep only reorders/permutes indices (sharding bookkeeping); all
feature compute runs on the device.
"""
import sys

sys.path.insert(0, "/opt/trn_rl_repo")

import numpy as np
from contextlib import ExitStack

from concourse import bass, mybir, tile, bacc, bass_utils
from concourse.masks import make_identity

P = 128                 # partitions
D = 64                  # channels
NCORES = 8
N_NODES = 100000
SHARD_N = 12544         # nodes per core incl pad (98 groups of 128)
NGROUPS = SHARD_N // P  # 98
SG = 7                  # groups per supergroup (gather call granularity)
NSG = NGROUPS // SG     # 14 supergroups
NTAB = SHARD_N * NCORES  # 100352 table rows
CHUNK = SHARD_N * 2     # 25088 rows per int16-addressable table chunk
NCHUNK = NTAB // CHUNK  # 4
F32 = mybir.dt.float32
BF16 = mybir.dt.bfloat16
# zero row inside each chunk: first pad node of the chunk's first shard,
# pad nodes are local ids [12500, 12544) -> (p = n % 128, g = n // 128);
# table row within shard = p * NGROUPS + g
_ZP, _ZG = 12500 % P, 12500 // P
Z_REL = _ZP * NGROUPS + _ZG  # relative row of a guaranteed-zero table row


# ---------------------------------------------------------------- host prep

def _prep(edge_index):
    """Pure-numpy sharding/bookkeeping. Returns per-core input arrays plus
    the shared (identical across cores) instruction-structure metadata."""
    src = np.asarray(edge_index[0], dtype=np.int64)
    dst = np.asarray(edge_index[1], dtype=np.int64)
    deg = np.bincount(dst, minlength=N_NODES)

    # global degree sort, round-robin over cores -> near-identical degree
    # profiles per core (keeps the shared padded lengths tight)
    order = np.argsort(deg, kind="stable")          # rank -> node
    rank = np.empty(N_NODES, np.int64)
    rank[order] = np.arange(N_NODES)
    node_core = rank % NCORES
    node_local = rank // NCORES                     # local id in [0, 12500)

    loc_p = node_local % P
    loc_g = node_local // P
    # global table row (partition-major within shard)
    tabrow = node_core * SHARD_N + loc_p * NGROUPS + loc_g

    e_core = node_core[dst]
    e_chunk = tabrow[src] // CHUNK
    e_group = loc_g[dst]
    e_q = loc_p[dst]          # dst slot within group (S column)
    e_rel = (tabrow[src] % CHUNK).astype(np.int64)

    # sort edges by (core, chunk, group, q, src) -- one lexsort
    sort_key = np.lexsort((e_rel, e_q, e_group, e_chunk, e_core))
    e_core = e_core[sort_key]
    e_chunk = e_chunk[sort_key]
    e_group = e_group[sort_key]
    e_q = e_q[sort_key]
    e_rel = e_rel[sort_key]

    # per (core, chunk, group) counts -> shared padded lengths
    cg_id = (e_core * NCHUNK + e_chunk) * NGROUPS + e_group
    counts = np.bincount(cg_id, minlength=NCORES * NCHUNK * NGROUPS).reshape(
        NCORES, NCHUNK, NGROUPS
    )
    lens = counts.max(axis=0)                       # [NCHUNK, NGROUPS]
    lens = ((lens + P - 1) // P) * P                # pad to window multiples
    lens = np.maximum(lens, P)                      # at least one window

    # slot layout: for core c, order (chunk, sg, group) concatenated
    # per-(c,chunk,group) slot range
    n_slots = int(lens.sum())                       # per core, per layer
    idx_all = np.full((NCORES, n_slots), Z_REL, np.int32)
    q_all = np.zeros((NCORES, n_slots), np.int32)

    # compute slot offsets per (chunk, group): chunk-major, then group
    slot_off = np.zeros((NCHUNK, NGROUPS), np.int64)
    run = 0
    for c in range(NCHUNK):
        for g in range(NGROUPS):
            slot_off[c, g] = run
            run += lens[c, g]
    assert run == n_slots

    # scatter edges into slots (vectorized per core)
    for core in range(NCORES):
        m = e_core == core
        ch, gr, qq, rel = e_chunk[m], e_group[m], e_q[m], e_rel[m]
        # position within its (chunk, group) run: edges are sorted, use
        # cumcount via groupby on (ch, gr)
        key = ch * NGROUPS + gr
        # cumulative index within equal-key runs (keys are sorted)
        start_of_run = np.r_[True, key[1:] != key[:-1]]
        run_starts = np.flatnonzero(start_of_run)
        within = np.arange(key.size) - np.repeat(run_starts, np.diff(np.r_[run_starts, key.size]))
        pos = slot_off[ch, gr] + within
        idx_all[core, pos] = rel.astype(np.int32)
        q_all[core, pos] = qq
        # pad slots keep q of last real edge in their window? q=0 is fine:
        # they gather the zero row, contributing nothing to column 0.

    # windows: n_slots/P of them; off values = q per slot (S column)
    n_win = n_slots // P

    # per-core gather index array in dma_gather wrap layout.
    # call granularity: (chunk, supergroup). slots of groups
    # [sg*SG,(sg+1)*SG) for chunk c are contiguous by construction.
    call_len = np.zeros((NCHUNK, NSG), np.int64)
    for c in range(NCHUNK):
        for s in range(NSG):
            call_len[c, s] = lens[c, s * SG : (s + 1) * SG].sum()

    # idx wrapped: position i -> [i % 16, i // 16], per call, calls packed
    # sequentially along columns; replicate the 16-row pattern to 128 rows.
    idx_wrap = np.empty((NCORES, 16, n_slots // 16), np.int16)
    col0 = np.zeros((NCHUNK, NSG), np.int64)
    run = 0
    for c in range(NCHUNK):
        for s in range(NSG):
            L = int(call_len[c, s])
            seg = idx_all[:, run : run + L]                # [NCORES, L]
            col0[c, s] = run // 16
            idx_wrap[:, :, run // 16 : (run + L) // 16] = seg.reshape(
                NCORES, L // 16, 16
            ).transpose(0, 2, 1)
            run += L
    assert run == n_slots

    # off (S column values) per window as f32 [P, n_win]: slot i of window w
    # sits at partition i % P
    off_f32 = q_all.reshape(NCORES, n_win, P).transpose(0, 2, 1).astype(np.float32)

    # per-node scale = 1/max(deg,1), at [p, g]
    scale = np.zeros((NCORES, P, NGROUPS), np.float32)
    invdeg = 1.0 / np.maximum(deg, 1)
    cores_n, p_n, g_n = node_core, loc_p, loc_g
    scale[cores_n, p_n, g_n] = invdeg
    # pad nodes (local >= 12500) keep scale 0

    # output mask for last group (zero out pad nodes so Z rows stay zero)
    outmask = np.ones((P, 1), np.float32)
    outmask[_ZP:, 0] = 0.0

    meta = {
        "lens": lens,                # [NCHUNK, NGROUPS] shared slot lens
        "slot_off": slot_off,        # slot offset of (chunk, group)
        "call_len": call_len,        # [NCHUNK, NSG]
        "col0": col0,                # idx column offset of call
        "n_slots": n_slots,
        "n_win": n_win,
    }
    percore = {
        "idx_wrap": idx_wrap,        # [NCORES, 16, n_slots//16] int16
        "off": off_f32,              # [NCORES, P, n_win] f32
        "scale": scale,              # [NCORES, P, NGROUPS] f32
        "outmask": outmask,          # [P, 1] f32 (same all cores)
        "node_core": node_core,
        "node_local": node_local,
    }
    return meta, percore


# ------------------------------------------------------------- bass program

def _build(meta):
    import os
    STAGE = int(os.environ.get("BIS_STAGE", "4"))
    MAXSG = int(os.environ.get("BIS_MAXSG", "99"))
    lens = meta["lens"]
    call_len = meta["call_len"]
    col0 = meta["col0"]
    n_slots = meta["n_slots"]
    n_win = meta["n_win"]

    nc = bacc.Bacc(
        "TRN2", target_bir_lowering=False, debug=False,
        num_devices=1 if os.environ.get("BIS_SIM") else NCORES,
        num_swdge_queues=4,
    )
    # ---- I/O
    x_shard = nc.dram_tensor("x_shard", [SHARD_N, 2 * D], BF16, kind="ExternalInput")
    idx_in = nc.dram_tensor("idx_in", [16, n_slots // 16], mybir.dt.int16, kind="ExternalInput")
    off_in = nc.dram_tensor("off_in", [P, n_win], F32, kind="ExternalInput")
    scale_in = nc.dram_tensor("scale_in", [P, NGROUPS], F32, kind="ExternalInput")
    mask_in = nc.dram_tensor("mask_in", [P, 1], F32, kind="ExternalInput")
    wl_in = [nc.dram_tensor(f"wl{i}", [D + 1, D], BF16, kind="ExternalInput") for i in range(2)]
    wr_in = [nc.dram_tensor(f"wr{i}", [D, D], BF16, kind="ExternalInput") for i in range(2)]
    a_in = [nc.dram_tensor(f"a{i}", [P, D], F32, kind="ExternalInput") for i in range(2)]
    out_ext = nc.dram_tensor("out", [SHARD_N, D], F32, kind="ExternalOutput")

    with tile.TileContext(nc) as tc:
        with ExitStack() as ctx:
            dram = ctx.enter_context(tc.tile_pool(name="dram", bufs=1, space="DRAM"))
            const = ctx.enter_context(tc.tile_pool(name="const", bufs=1))
            gath = ctx.enter_context(tc.tile_pool(name="gath", bufs=2))
            spool = ctx.enter_context(tc.tile_pool(name="spool", bufs=6))
            epi = ctx.enter_context(tc.tile_pool(name="epi", bufs=6))
            ps_agg = ctx.enter_context(tc.tile_pool(name="ps_agg", bufs=2, space="PSUM"))
            ps_tp = ctx.enter_context(tc.tile_pool(name="ps_tp", bufs=2, space="PSUM"))
            ps_h = ctx.enter_context(tc.tile_pool(name="ps_h", bufs=4, space="PSUM"))

            # ---- persistent SBUF state
            idx_sb = const.tile([P, n_slots // 16], mybir.dt.int16)
            # replicate 16-row wrap pattern across all 128 partitions
            for rep in range(8):
                nc.sync.dma_start(idx_sb[16 * rep : 16 * (rep + 1), :], idx_in[:])
            off_sb = const.tile([P, n_win], F32)
            nc.sync.dma_start(off_sb[:], off_in[:])
            scale_sb = const.tile([P, NGROUPS], F32)
            nc.sync.dma_start(scale_sb[:], scale_in[:])
            mask_sb = const.tile([P, 1], F32)
            nc.sync.dma_start(mask_sb[:], mask_in[:])
            wl_sb = [const.tile([D + 1, D], BF16, tag=f"wl{i}", name=f"wl_sb{i}") for i in range(2)]
            wr_sb = [const.tile([D, D], BF16, tag=f"wr{i}", name=f"wr_sb{i}") for i in range(2)]
            a_sb = [const.tile([P, D], F32, tag=f"asb{i}", name=f"a_sb{i}") for i in range(2)]
            for i in range(2):
                nc.sync.dma_start(wl_sb[i][:], wl_in[i][:])
                nc.sync.dma_start(wr_sb[i][:], wr_in[i][:])
                nc.sync.dma_start(a_sb[i][:], a_in[i][:])
            ident = const.tile([P, P], BF16)
            make_identity(nc, ident[:])
            iota = const.tile([P, P], BF16)
            iota_i = const.tile([P, P], mybir.dt.int32)
            nc.gpsimd.iota(iota_i[:], pattern=[[1, P]], base=0, channel_multiplier=0)
            nc.vector.tensor_copy(iota[:], iota_i[:])

            # node features of this core's shard, [128, NGROUPS, 128] bf16
            # (rows padded to 256B so dma_gather can fetch them directly)
            x_sb = const.tile([P, NGROUPS * 2 * D], BF16)
            nc.sync.dma_start(
                x_sb[:], x_shard[:].rearrange("(p r) d -> p (r d)", p=P)
            )
            h_sb = const.tile([P, NGROUPS * 2 * D], BF16)
            # final f32 output reuses x_sb's bytes (x is dead by layer 2)
            out_view = x_sb[:].bitcast(F32)
            xT_all = const.tile([D, NGROUPS * P], BF16)
            if STAGE < 4:
                nc.vector.memset(h_sb[:], 0.0)

            # DRAM: AllGather bounce + tables
            ag_in = dram.tile([SHARD_N, 2 * D], BF16)
            REPEAT = int(os.environ.get("BIS_REPEAT", "1"))
            tabs = [
                dram.tile([NTAB, 2 * D], BF16, addr_space="Shared", tag=f"tab{i}", name=f"tab{i}")
                for i in range(2 * REPEAT)
            ]

            cur_sb = x_sb  # SBUF copy of this core's current features
            for layer2 in range(2 * REPEAT):
                layer = layer2 % 2
                tab = tabs[layer2]
                if layer == 0:
                    nc.sync.dma_start(ag_in[:], x_shard[:])
                else:
                    nc.sync.dma_start(
                        ag_in[:].rearrange("(p r) d -> p (r d)", p=P), cur_sb[:]
                    )
                if os.environ.get("BIS_SIM"):
                    # single-core cost-model sim: no collectives supported
                    nc.sync.dma_start(tab[0:SHARD_N, :], ag_in[:])
                else:
                    nc.gpsimd.collective_compute(
                        "AllGather",
                        mybir.AluOpType.bypass,
                        replica_groups=[list(range(NCORES))],
                        ins=[ag_in.opt()],
                        outs=[tab.opt()],
                    )

                # feature-major bf16 copy of this core's features, built
                # upfront (overlaps gathers); used by the w_r matmul
                for g in range(NGROUPS):
                    xb = epi.tile([P, D], BF16, tag="xb", name="xb")
                    nc.vector.tensor_copy(
                        xb[:],
                        cur_sb[:].rearrange("p (r d) -> p r d", d=2 * D)[:, g, :D],
                    )
                    xT_ps = ps_tp.tile([D, P], BF16, space="PSUM", tag="tp1", name="xT_ps")
                    nc.tensor.transpose(out=xT_ps[:], in_=xb[:], identity=ident[:])
                    nc.vector.tensor_copy(xT_all[:, g * P : (g + 1) * P], xT_ps[:])

                win_id = 0  # global window counter this layer
                for sg in range(NSG):
                    g_lo = sg * SG
                    # gather all 4 chunks for this supergroup
                    ftiles = []
                    for c in range(NCHUNK):
                        if STAGE < 1 or sg >= MAXSG:
                            continue
                        L = int(call_len[c, sg])
                        ftb = gath.tile([P, (L // P) * 2 * D], BF16, tag=f"ftb{c}", name=f"ftb{c}")
                        # split calls over MAXIDX (descriptor-ring limit)
                        MAXIDX = 8192
                        done = 0
                        while done < L:
                            n = min(MAXIDX, L - done)
                            nc.gpsimd.dma_gather(
                                out_ap=ftb[:].rearrange("p (w d) -> p w d", d=2 * D)[
                                    :, done // P : (done + n) // P, :
                                ],
                                in_ap=tab[c * CHUNK : (c + 1) * CHUNK, :],
                                idxs_ap=idx_sb[
                                    :,
                                    int(col0[c, sg]) + done // 16 : int(col0[c, sg])
                                    + (done + n) // 16,
                                ],
                                num_idxs=n,
                                num_idxs_reg=n,
                                elem_size=2 * D,
                                single_packet=False,
                                queue_num=c,
                            )
                            done += n
                        ftiles.append(ftb)

                    for gg in range(SG):
                        g = g_lo + gg
                        if STAGE < 3:
                            continue
                        psum = ps_agg.tile([P, D], F32, space="PSUM")
                        # windows of group g across the 4 chunks
                        total_w = int(sum(lens[c, g] for c in range(NCHUNK)) // P)
                        wdone = 0
                        for c in range(NCHUNK):
                            nw = int(lens[c, g] // P)
                            # column offset of group g inside this chunk tile
                            cbase = int(
                                (meta["slot_off"][c, g] - meta["slot_off"][c, g_lo])
                                // P
                            )
                            for w in range(nw):
                                if STAGE < 2:
                                    win_id += 1
                                    wdone += 1
                                    continue
                                s_t = spool.tile([P, P], BF16, tag="s")
                                eng = nc.vector
                                off_col = int(meta["slot_off"][c, g]) // P + w
                                if STAGE >= 2:
                                    eng.tensor_scalar(
                                        out=s_t[:],
                                        in0=iota[:],
                                        scalar1=off_sb[:, off_col : off_col + 1],
                                        scalar2=None,
                                        op0=mybir.AluOpType.is_equal,
                                    )
                                if STAGE >= 3:
                                    nc.tensor.matmul(
                                        psum[:],
                                        lhsT=s_t[:],
                                        rhs=ftiles[c][:].rearrange(
                                            "p (w d) -> p w d", d=2 * D
                                        )[:, cbase + w, :D],
                                        start=(wdone == 0),
                                        stop=(wdone == total_w - 1),
                                    )
                                win_id += 1
                                wdone += 1

                        # ---- epilogue for group g
                        if STAGE < 4:
                            continue
                        # scaled mean + ones column for fused bias
                        sagg = epi.tile([P, D + 1], BF16, tag="sagg")
                        nc.vector.tensor_scalar(
                            out=sagg[:, :D],
                            in0=psum[:],
                            scalar1=scale_sb[:, g : g + 1],
                            scalar2=None,
                            op0=mybir.AluOpType.mult,
                        )
                        nc.gpsimd.memset(sagg[:, D : D + 1], 1.0)
                        # transpose [P, 65] -> [65, P]
                        saggT_ps = ps_tp.tile([D + 1, P], BF16, space="PSUM", tag="tp1", name="saggT_ps")
                        nc.tensor.transpose(
                            out=saggT_ps[:], in_=sagg[:], identity=ident[:]
                        )
                        saggT = epi.tile([D + 1, P], BF16, tag="saggT")
                        nc.vector.tensor_copy(saggT[:], saggT_ps[:])

                        # h = sagg @ wl(+bias row) + x @ wr
                        h_ps = ps_h.tile([P, D], F32, space="PSUM")
                        nc.tensor.matmul(
                            h_ps[:], lhsT=saggT[:], rhs=wl_sb[layer][:],
                            start=True, stop=False,
                        )
                        nc.tensor.matmul(
                            h_ps[:], lhsT=xT_all[:, g * P : (g + 1) * P],
                            rhs=wr_sb[layer][:],
                            start=False, stop=True,
                        )
                        # PReLU: h = relu(v) + a * (v - relu(v))
                        pos = epi.tile([P, D], F32, tag="pos")
                        nc.scalar.activation(
                            out=pos[:], in_=h_ps[:],
                            func=mybir.ActivationFunctionType.Relu,
                        )
                        neg = epi.tile([P, D], F32, tag="neg")
                        nc.vector.tensor_sub(neg[:], h_ps[:], pos[:])
                        nc.vector.tensor_mul(neg[:], neg[:], a_sb[layer][:])
                        if layer == 0:
                            hview = h_sb[:].rearrange(
                                "p (r d) -> p r d", d=2 * D
                            )[:, g, :D]
                        else:
                            hview = out_view.rearrange(
                                "p (r d) -> p r d", d=D
                            )[:, g, :]
                        if g == NGROUPS - 1:
                            nc.vector.tensor_add(neg[:], neg[:], pos[:])
                            nc.vector.tensor_scalar(
                                out=hview, in0=neg[:],
                                scalar1=mask_sb[:, 0:1], scalar2=None,
                                op0=mybir.AluOpType.mult,
                            )
                        else:
                            nc.vector.tensor_add(hview, neg[:], pos[:])

                # next layer reads from h_sb; the final layer wrote out_sb
                cur_sb = h_sb

            nc.sync.dma_start(
                out_ext[:].rearrange("(p r) d -> p (r d)", p=P), out_view
            )

    nc.compile()
    return nc


# ------------------------------------------------------------------ runner

_CACHE = {}


def _get_program(edge_index):
    key = hash(
        (edge_index.shape, edge_index.dtype.str, edge_index[:, ::997].tobytes())
    )
    if key not in _CACHE:
        meta, percore = _prep(edge_index)
        nc = _build(meta)
        _CACHE[key] = (nc, meta, percore)
    return _CACHE[key]


def kernel(x, edge_index, w_l0, b_l0, w_r0, a0, w_l1, b_l1, w_r1, a1):
    x = np.asarray(x, dtype=np.float32)
    edge_index = np.asarray(edge_index)
    nc, meta, pc = _get_program(edge_index)

    node_core = pc["node_core"]
    node_local = pc["node_local"]

    # build permuted per-core x shards in table layout (row = p*NGROUPS + g)
    loc_p = node_local % P
    loc_g = node_local // P
    import ml_dtypes
    xs = np.zeros((NCORES, SHARD_N, 2 * D), ml_dtypes.bfloat16)
    xs[node_core, loc_p * NGROUPS + loc_g, :D] = x.astype(ml_dtypes.bfloat16)

    wls = []
    for wl, bl in ((w_l0, b_l0), (w_l1, b_l1)):
        wls.append(
            np.concatenate(
                [np.asarray(wl, np.float32), np.asarray(bl, np.float32)[None, :]], 0
            ).astype(ml_dtypes.bfloat16)
        )
    import ml_dtypes as _mld
    ar = [
        np.repeat(np.asarray(a, np.float32)[None, :], P, 0) for a in (a0, a1)
    ]
    wrs = [np.asarray(w, np.float32).astype(ml_dtypes.bfloat16) for w in (w_r0, w_r1)]

    in_maps = []
    for c in range(NCORES):
        idxw = pc["idx_wrap"][c]
        in_maps.append(
            {
                "x_shard": xs[c],
                "idx_in": idxw,
                "off_in": pc["off"][c],
                "scale_in": pc["scale"][c],
                "mask_in": pc["outmask"],
                "wl0": wls[0],
                "wl1": wls[1],
                "wr0": wrs[0],
                "wr1": wrs[1],
                "a0": ar[0],
                "a1": ar[1],
            }
        )

    global _last_in_maps
    _last_in_maps = in_maps
    res = bass_utils.run_bass_kernel_spmd(nc, in_maps, core_ids=list(range(NCORES)))

    out = np.empty((N_NODES, D), np.float32)
    for c in range(NCORES):
        shard_out = res.results[c]["out"]            # [SHARD_N, D]
        m = node_core == c
        out[m] = shard_out[loc_p[m] * NGROUPS + loc_g[m]]
    return out

